# revision 6
# baseline (speedup 1.0000x reference)
"""Trainium2 Bass kernel for nn_KGRL (KG-based RL dialog model forward pass).

Strategy (8 NeuronCores):
  - Data-parallel over batch B=8192 -> B_loc=1024 per core for everything
    downstream of graph_rep.
  - graph_rep = relu(kg_adj @ W_g) is batch-independent: row-shard kg_adj
    (512 rows/core), compute the graph_rep shard on each core, AllGather
    to the full [4096, 256] graph_rep.
  - PE (TensorE) transposes put the contraction dim on partitions where
    needed (dsr, kg_adj, state_, sym_mask, disease_dis).
  - Matmuls with free dim >= 256 run in float32r (4-xbus fp32 streaming,
    4x faster than plain fp32 on TRN2).
  - Free-dim biases are folded into the matmuls via an appended ones-row
    (lhsT=[1,128] ones, rhs=[1,N] bias row).
"""

import numpy as np

import concourse.bacc as bacc
import concourse.bass as bass
import concourse.mybir as mybir
import concourse.tile as tile
from concourse.bass_utils import run_bass_kernel_spmd
from concourse.masks import make_identity

# Problem dims (hardcoded per contract)
B, E, D, HID, DIS, SYM = 8192, 4096, 256, 1024, 128, 512
DSZ = DIS + SYM  # 640
N_CORES = 8
B_LOC = B // N_CORES  # 1024
E_LOC = E // N_CORES  # 512
P = 128

F32 = mybir.dt.float32
F32R = mybir.dt.float32r
I32 = mybir.dt.int32

USE_F32R = True


def _r(ap):
    """Tag an fp32 AP for float32r (fast fp32) matmul streaming."""
    return ap.bitcast(F32R) if USE_F32R else ap


def build_nc():
    nc = bacc.Bacc(
        "TRN2",
        target_bir_lowering=False,
        debug=False,
        enable_asserts=True,
        num_devices=N_CORES,
    )

    # ---- DRAM I/O (per-core shards; weights replicated) ----
    kga = nc.dram_tensor("kga", [E_LOC, E], F32, kind="ExternalInput")
    wg = nc.dram_tensor("wg", [E, D], F32, kind="ExternalInput")
    dsr = nc.dram_tensor("dsr", [B_LOC, E], F32, kind="ExternalInput")
    ds = nc.dram_tensor("ds", [B_LOC, DSZ], I32, kind="ExternalInput")
    symmask = nc.dram_tensor("symmask", [B_LOC, SYM], I32, kind="ExternalInput")
    symflag = nc.dram_tensor("symflag", [B_LOC, SYM], I32, kind="ExternalInput")
    sympmask = nc.dram_tensor("sympmask", [B_LOC, SYM], I32, kind="ExternalInput")
    dmask = nc.dram_tensor("dmask", [B_LOC, DIS], I32, kind="ExternalInput")
    wd1 = nc.dram_tensor("wd1", [SYM, DIS], F32, kind="ExternalInput")
    bd1 = nc.dram_tensor("bd1", [DIS], F32, kind="ExternalInput")
    w1 = nc.dram_tensor("w1", [D, HID], F32, kind="ExternalInput")
    b1 = nc.dram_tensor("b1", [HID], F32, kind="ExternalInput")
    w2 = nc.dram_tensor("w2", [HID, SYM], F32, kind="ExternalInput")
    b2 = nc.dram_tensor("b2", [SYM], F32, kind="ExternalInput")
    wm1 = nc.dram_tensor("wm1", [DSZ, HID], F32, kind="ExternalInput")
    bm1 = nc.dram_tensor("bm1", [HID], F32, kind="ExternalInput")
    wm2 = nc.dram_tensor("wm2", [HID, 1], F32, kind="ExternalInput")
    bm2 = nc.dram_tensor("bm2", [1], F32, kind="ExternalInput")
    md = nc.dram_tensor("md", [DIS, SYM], F32, kind="ExternalInput")

    out_dd = nc.dram_tensor("out_dd", [B_LOC, DIS], F32, kind="ExternalOutput")
    out_sf = nc.dram_tensor("out_sf", [B_LOC, SYM], F32, kind="ExternalOutput")

    AG = mybir.ActivationFunctionType
    ALU = mybir.AluOpType
    AX = mybir.AxisListType

    with tile.TileContext(nc) as tc:
        evict_cnt = [0]

        def evict(dst, src):
            # alternate psum->sbuf copies between DVE and ACT (2:1)
            evict_cnt[0] += 1
            if evict_cnt[0] % 3 == 0:
                nc.scalar.copy(dst, src)
            else:
                nc.vector.tensor_copy(dst, src)

        with (
            tc.tile_pool(name="const", bufs=1) as constp,
            tc.tile_pool(name="persist", bufs=1) as persist,
            tc.tile_pool(name="dram", bufs=1, space="DRAM") as dramp,
        ):
            ident_f = constp.tile([P, P], F32)
            make_identity(nc, ident_f)
            ident = constp.tile([P, P], F32R)
            nc.vector.tensor_copy(ident[:], ident_f[:])
            ones_row = constp.tile([1, P], F32)
            nc.vector.memset(ones_row[:], 1.0)
            ones_r = constp.tile([1, P], F32R)
            nc.vector.tensor_copy(ones_r[:], ones_row[:])

            # stateT [256, 1024]: chunk m (of 2) at cols m*1024:(m+1)*1024
            stateT = persist.tile([P, 2 * B_LOC], F32R)

            grep_shard_d = dramp.tile([E_LOC, D], F32)
            grep_full_d = dramp.tile([E, D], F32, addr_space="Shared")

            # ================= Phase A: graph_rep shard + AllGather ========
            with (
                tc.tile_pool(name="wgp", bufs=1) as wgp,
                tc.tile_pool(name="kap", bufs=2) as kap,
                tc.tile_pool(name="katp", bufs=3) as katp,
                tc.tile_pool(name="psA", bufs=2, space="PSUM") as psA,
                tc.tile_pool(name="psAg", bufs=2, space="PSUM") as psAg,
                tc.tile_pool(name="gap", bufs=2) as gap,
            ):
                wg_sb = wgp.tile([P, 32 * D], F32R)
                nc.sync.dma_start(
                    out=wg_sb[:].rearrange("p (k d) -> p k d", d=D),
                    in_=_r(wg.ap().rearrange("(k p) d -> p k d", p=P)),
                )
                wg3 = wg_sb[:].rearrange("p (k d) -> p k d", d=D)

                for m in range(4):
                    kam = kap.tile([P, E], F32R, name="kam")
                    nc.sync.dma_start(out=kam[:], in_=_r(kga.ap()[m * P:(m + 1) * P, :]))
                    psg = psAg.tile([P, D], F32, name="psg")
                    for k4 in range(8):
                        pstA = psA.tile([P, 4 * P], F32, name="pstA")
                        for j in range(4):
                            kk = k4 * 4 + j
                            nc.tensor.transpose(
                                _r(pstA[:, j * P:(j + 1) * P]),
                                _r(kam[:, kk * P:(kk + 1) * P]),
                                _r(ident[:]),
                            )
                        kaT = katp.tile([P, 4 * P], F32R, name="kaT")
                        evict(kaT[:], pstA[:])
                        for j in range(4):
                            kk = k4 * 4 + j
                            nc.tensor.matmul(
                                psg[:],
                                _r(kaT[:, j * P:(j + 1) * P]),
                                _r(wg3[:, kk, :]),
                                start=(kk == 0),
                                stop=(kk == 31),
                                skip_group_check=True,
                            )
                    gsb = gap.tile([P, D], F32, name="gsb")
                    nc.scalar.activation(gsb[:], psg[:], AG.Relu)
                    nc.sync.dma_start(
                        out=grep_shard_d[m * P:(m + 1) * P, :], in_=gsb[:]
                    )

                nc.gpsimd.collective_compute(
                    "AllGather",
                    ALU.bypass,
                    replica_groups=[list(range(N_CORES))],
                    ins=[grep_shard_d.opt()],
                    outs=[grep_full_d.opt()],
                )

            # ================= Phase B: stateT = (dsr @ graph_rep).T =======
            with (
                tc.tile_pool(name="grp", bufs=1) as grp,
                tc.tile_pool(name="dsp", bufs=1) as dsp,
                tc.tile_pool(name="dTp", bufs=3) as dTp,
                tc.tile_pool(name="psBt", bufs=2, space="PSUM") as psBt,
                tc.tile_pool(name="psBa", bufs=2, space="PSUM") as psBa,
            ):
                grep_sb = grp.tile([P, 32 * D], F32R)
                nc.sync.dma_start(
                    out=grep_sb[:].rearrange("p (k d) -> p k d", d=D),
                    in_=_r(grep_full_d[:].rearrange("(k p) d -> p k d", p=P)),
                )
                gr3 = grep_sb[:].rearrange("p (k d) -> p k d", d=D)

                for nb in range(2):
                    dsr_t = []
                    for c in range(4):
                        dt_ = dsp.tile([P, E], F32R, name=f"dsr{c}", tag=f"dsr{c}")
                        nc.sync.dma_start(
                            out=dt_[:],
                            in_=_r(dsr.ap()[(nb * 4 + c) * P:(nb * 4 + c + 1) * P, :]),
                        )
                        dsr_t.append(dt_)
                    ps0 = psBa.tile([P, 512], F32, name="ps0")
                    ps1 = psBa.tile([P, 512], F32, name="ps1")
                    for k in range(32):
                        pstB = psBt.tile([P, 512], F32, name="pstB")
                        for c in range(4):
                            nc.tensor.transpose(
                                _r(pstB[:, c * P:(c + 1) * P]),
                                _r(dsr_t[c][:, k * P:(k + 1) * P]),
                                _r(ident[:]),
                            )
                        dT = dTp.tile([P, 512], F32R, name="dT")
                        evict(dT[:], pstB[:])
                        nc.tensor.matmul(
                            ps0[:], _r(gr3[:, k, 0:P]), _r(dT[:]),
                            start=(k == 0), stop=(k == 31), skip_group_check=True,
                        )
                        nc.tensor.matmul(
                            ps1[:], _r(gr3[:, k, P:2 * P]), _r(dT[:]),
                            start=(k == 0), stop=(k == 31), skip_group_check=True,
                        )
                    evict(stateT[:, nb * 512:(nb + 1) * 512], ps0[:])
                    evict(stateT[:, B_LOC + nb * 512:B_LOC + (nb + 1) * 512], ps1[:])

            # ================= Phase C: MLPs + elementwise =================
            with (
                tc.tile_pool(name="wc", bufs=1) as wc,
                tc.tile_pool(name="stT", bufs=1) as stTp,
                tc.tile_pool(name="syT", bufs=1) as syTp,
                tc.tile_pool(name="hp", bufs=1) as hp,
                tc.tile_pool(name="cw", bufs=2) as cw,
                tc.tile_pool(name="cw5", bufs=2) as cw5,
                tc.tile_pool(name="pst", bufs=2, space="PSUM") as pst,
                tc.tile_pool(name="psmm", bufs=2, space="PSUM") as psmm,
                tc.tile_pool(name="pssm", bufs=2, space="PSUM") as pssm,
            ):
                # --- replicated weights/biases to SBUF ---
                wm1_sb = wc.tile([P, 5 * HID], F32R)
                nc.sync.dma_start(
                    out=wm1_sb[:].rearrange("p (k h) -> p k h", h=HID),
                    in_=_r(wm1.ap().rearrange("(k p) h -> p k h", p=P)),
                )
                wm13 = wm1_sb[:].rearrange("p (k h) -> p k h", h=HID)
                w1_sb = wc.tile([P, 2 * HID], F32R)
                nc.sync.dma_start(
                    out=w1_sb[:].rearrange("p (k h) -> p k h", h=HID),
                    in_=_r(w1.ap().rearrange("(k p) h -> p k h", p=P)),
                )
                w13 = w1_sb[:].rearrange("p (k h) -> p k h", h=HID)
                w2_sb = wc.tile([P, 8 * SYM], F32R)
                nc.sync.dma_start(
                    out=w2_sb[:].rearrange("p (k s) -> p k s", s=SYM),
                    in_=_r(w2.ap().rearrange("(k p) s -> p k s", p=P)),
                )
                w23 = w2_sb[:].rearrange("p (k s) -> p k s", s=SYM)
                wd1_sb = wc.tile([P, 4 * DIS], F32)
                nc.sync.dma_start(
                    out=wd1_sb[:].rearrange("p (k d) -> p k d", d=DIS),
                    in_=wd1.ap().rearrange("(k p) d -> p k d", p=P),
                )
                wd13 = wd1_sb[:].rearrange("p (k d) -> p k d", d=DIS)
                md_sb = wc.tile([P, SYM], F32R)
                nc.sync.dma_start(out=md_sb[:], in_=_r(md.ap()[:, :]))
                wm2_sb = wc.tile([P, 8], F32)
                nc.sync.dma_start(
                    out=wm2_sb[:],
                    in_=wm2.ap().rearrange("(k p) o -> p (k o)", p=P),
                )
                bm1_sb = wc.tile([P, 8], F32)
                nc.sync.dma_start(
                    out=bm1_sb[:], in_=bm1.ap().rearrange("(m p) -> p m", p=P)
                )
                b1_sb = wc.tile([P, 8], F32)
                nc.sync.dma_start(
                    out=b1_sb[:], in_=b1.ap().rearrange("(m p) -> p m", p=P)
                )
                b2_sb = wc.tile([1, SYM], F32R)
                nc.sync.dma_start(
                    out=b2_sb[:], in_=_r(b2.ap().rearrange("(o s) -> o s", o=1))
                )
                bd1_sb = wc.tile([1, DIS], F32)
                nc.sync.dma_start(
                    out=bd1_sb[:], in_=bd1.ap().rearrange("(o d) -> o d", o=1)
                )
                bm2_sb = wc.tile([1, 1], F32)
                nc.sync.dma_start(
                    out=bm2_sb[:], in_=bm2.ap().rearrange("(o d) -> o d", o=1)
                )
                mu_sb = wc.tile([P, 8], F32)

                for nb in range(2):
                    stT = stTp.tile([P, 5 * 512], F32R, name="stT")
                    stT3 = stT[:].rearrange("p (k b) -> p k b", b=512)
                    syT = syTp.tile([P, 4 * 512], F32, name="syT")
                    syT3 = syT[:].rearrange("p (k b) -> p k b", b=512)

                    # --- per-chunk: state_, transposes of state_ and symf ---
                    for c4 in range(4):
                        c = nb * 4 + c4
                        dsi = cw5.tile([P, DSZ], I32, name="dsi")
                        nc.sync.dma_start(out=dsi[:], in_=ds.ap()[c * P:(c + 1) * P, :])
                        dsf = cw5.tile([P, DSZ], F32, name="dsf")
                        nc.vector.tensor_copy(dsf[:], dsi[:])
                        st_ = cw5.tile([P, DSZ], F32R, name="st_")
                        nc.vector.tensor_scalar(st_[:], dsf[:], 1.0, None, ALU.is_equal)

                        # transpose state_ -> stT (5 k-chunks)
                        pstC = pst.tile([P, 512], F32, name="pstC", tag="tp")
                        for sk in range(4):
                            nc.tensor.transpose(
                                _r(pstC[:, sk * P:(sk + 1) * P]),
                                _r(st_[:, sk * P:(sk + 1) * P]),
                                _r(ident[:]),
                            )
                        evict(
                            stT3[:, 0:4, c4 * P:(c4 + 1) * P],
                            pstC[:].rearrange("p (k b) -> p k b", b=P),
                        )
                        pstC2 = pst.tile([P, 512], F32, name="pstC2", tag="tp")
                        nc.tensor.transpose(
                            _r(pstC2[:, 0:P]), _r(st_[:, 4 * P:5 * P]), _r(ident[:])
                        )
                        evict(stT3[:, 4, c4 * P:(c4 + 1) * P], pstC2[:, 0:P])

                        # symf cast + transpose -> syT (4 k-chunks)
                        syi = cw5.tile([P, SYM], I32, name="syi")
                        nc.sync.dma_start(
                            out=syi[:], in_=symmask.ap()[c * P:(c + 1) * P, :]
                        )
                        syf = cw5.tile([P, SYM], F32R, name="syf")
                        nc.vector.tensor_copy(syf[:], syi[:])
                        pstS = pst.tile([P, 512], F32, name="pstS", tag="tp")
                        for sk in range(4):
                            nc.tensor.transpose(
                                _r(pstS[:, sk * P:(sk + 1) * P]),
                                _r(syf[:, sk * P:(sk + 1) * P]),
                                _r(ident[:]),
                            )
                        evict(
                            syT3[:, 0:4, c4 * P:(c4 + 1) * P],
                            pstS[:].rearrange("p (k b) -> p k b", b=P),
                        )

                    # --- mu path: hidden_muT then mu ---
                    hmuT = hp.tile([P, 8 * 512], F32R, name="hmuT", tag="hbig")
                    for m in range(8):
                        psh = psmm.tile([P, 512], F32, name="psh", tag="mm")
                        for k in range(5):
                            nc.tensor.matmul(
                                psh[:],
                                _r(wm13[:, k, m * P:(m + 1) * P]),
                                _r(stT3[:, k, :]),
                                start=(k == 0),
                                stop=(k == 4),
                                skip_group_check=True,
                            )
                        nc.scalar.activation(
                            hmuT[:, m * 512:(m + 1) * 512], psh[:], AG.Relu,
                            bias=bm1_sb[:, m:m + 1],
                        )
                    for c4 in range(4):
                        c = nb * 4 + c4
                        psmu = pssm.tile([P, DIS], F32, name="psmu", tag="sm")
                        for m in range(8):
                            nc.tensor.matmul(
                                psmu[:, 0:1],
                                hmuT[:, m * 512 + c4 * P:m * 512 + (c4 + 1) * P].bitcast(F32),
                                wm2_sb[:, m:m + 1],
                                start=(m == 0),
                                stop=False,
                                skip_group_check=True,
                            )
                        nc.tensor.matmul(
                            psmu[:, 0:1], ones_row[0:1, :], bm2_sb[0:1, :],
                            start=False, stop=True, skip_group_check=True,
                        )
                        nc.scalar.activation(mu_sb[:, c:c + 1], psmu[:, 0:1], AG.Sigmoid)

                    # --- symptom hidden (needs stateT from phase B) ---
                    hT = hp.tile([P, 8 * 512], F32R, name="hT", tag="hbig")
                    for m in range(8):
                        psh2 = psmm.tile([P, 512], F32, name="psh2", tag="mm")
                        for k in range(2):
                            nc.tensor.matmul(
                                psh2[:],
                                _r(w13[:, k, m * P:(m + 1) * P]),
                                _r(stateT[:, k * B_LOC + nb * 512:k * B_LOC + (nb + 1) * 512]),
                                start=(k == 0),
                                stop=(k == 1),
                                skip_group_check=True,
                            )
                        nc.scalar.activation(
                            hT[:, m * 512:(m + 1) * 512], psh2[:], AG.Relu,
                            bias=b1_sb[:, m:m + 1],
                        )

                    # --- per-chunk: disease, tfidf, symptom_p, blend ---
                    for c4 in range(4):
                        c = nb * 4 + c4
                        # disease logits + softmax
                        psd = pssm.tile([P, DIS], F32, name="psd", tag="sm")
                        for sk in range(4):
                            nc.tensor.matmul(
                                psd[:],
                                syT3[:, sk, c4 * P:(c4 + 1) * P],
                                wd13[:, sk, :],
                                start=(sk == 0),
                                stop=False,
                                skip_group_check=True,
                            )
                        nc.tensor.matmul(
                            psd[:], ones_row[0:1, :], bd1_sb[0:1, :],
                            start=False, stop=True, skip_group_check=True,
                        )
                        nmx = cw.tile([P, 1], F32, name="nmx")
                        nc.vector.reduce_max(nmx[:], psd[:], axis=AX.X, negate=True)
                        ex = cw.tile([P, DIS], F32, name="ex")
                        nc.scalar.activation(ex[:], psd[:], AG.Exp, bias=nmx[:])
                        sm = cw.tile([P, 1], F32, name="sm")
                        nc.vector.reduce_sum(sm[:], ex[:], axis=AX.X)
                        rs = cw.tile([P, 1], F32, name="rs")
                        nc.vector.reciprocal(rs[:], sm[:])
                        dmi = cw.tile([P, DIS], I32, name="dmi")
                        nc.sync.dma_start(
                            out=dmi[:], in_=dmask.ap()[c * P:(c + 1) * P, :]
                        )
                        dmf = cw.tile([P, DIS], F32, name="dmf")
                        nc.vector.tensor_copy(dmf[:], dmi[:])
                        nc.vector.tensor_scalar(ex[:], ex[:], rs[:], None, ALU.mult)
                        dd_c = cw.tile([P, DIS], F32R, name="dd_c")
                        nc.vector.tensor_mul(dd_c[:], ex[:], dmf[:])
                        nc.sync.dma_start(
                            out=out_dd.ap()[c * P:(c + 1) * P, :], in_=dd_c[:].bitcast(F32)
                        )

                        # tfidf: transpose dd_c, matmul with m_d, filter-sigmoid
                        psdT = pst.tile([P, 512], F32, name="psdT", tag="tp")
                        nc.tensor.transpose(_r(psdT[:, 0:P]), _r(dd_c[:]), _r(ident[:]))
                        ddT = cw.tile([P, P], F32R, name="ddT")
                        evict(ddT[:], psdT[:, 0:P])
                        pstf = psmm.tile([P, SYM], F32, name="pstf", tag="mm")
                        nc.tensor.matmul(
                            pstf[:], _r(ddT[:]), _r(md_sb[:]),
                            start=True, stop=True, skip_group_check=True,
                        )
                        sig = cw.tile([P, SYM], F32, name="sig")
                        nc.scalar.activation(sig[:], pstf[:], AG.Sigmoid)
                        gt0 = cw.tile([P, SYM], I32, name="gt0")
                        nc.vector.tensor_scalar(gt0[:], pstf[:], 0.0, None, ALU.is_gt)
                        stf = cw.tile([P, SYM], F32, name="stf")
                        nc.vector.tensor_copy(stf[:], pstf[:])
                        nc.vector.copy_predicated(stf[:], gt0[:], sig[:])

                        # symptom_p logits + sigmoid
                        pssp = psmm.tile([P, SYM], F32, name="pssp", tag="mm")
                        for m in range(8):
                            nc.tensor.matmul(
                                pssp[:],
                                _r(hT[:, m * 512 + c4 * P:m * 512 + (c4 + 1) * P]),
                                _r(w23[:, m, :]),
                                start=(m == 0),
                                stop=False,
                                skip_group_check=True,
                            )
                        nc.tensor.matmul(
                            pssp[:], ones_r[0:1, :], b2_sb[0:1, :],
                            start=False, stop=True, skip_group_check=True,
                        )
                        sp = cw.tile([P, SYM], F32, name="sp")
                        nc.scalar.activation(sp[:], pssp[:], AG.Sigmoid)

                        # blend + final masks
                        diff = cw.tile([P, SYM], F32, name="diff")
                        nc.vector.tensor_sub(diff[:], sp[:], stf[:])
                        nc.vector.tensor_scalar(
                            diff[:], diff[:], mu_sb[:, c:c + 1], None, ALU.mult
                        )
                        spr = cw.tile([P, SYM], F32, name="spr")
                        nc.vector.tensor_add(spr[:], diff[:], stf[:])
                        fli = cw.tile([P, SYM], I32, name="fli")
                        nc.sync.dma_start(
                            out=fli[:], in_=symflag.ap()[c * P:(c + 1) * P, :]
                        )
                        pmi = cw.tile([P, SYM], I32, name="pmi")
                        nc.sync.dma_start(
                            out=pmi[:], in_=sympmask.ap()[c * P:(c + 1) * P, :]
                        )
                        fmi = cw.tile([P, SYM], I32, name="fmi")
                        nc.vector.tensor_mul(fmi[:], fli[:], pmi[:])
                        fmf = cw.tile([P, SYM], F32, name="fmf")
                        nc.vector.tensor_copy(fmf[:], fmi[:])
                        fin = cw.tile([P, SYM], F32, name="fin")
                        nc.vector.tensor_mul(fin[:], spr[:], fmf[:])
                        nc.sync.dma_start(
                            out=out_sf.ap()[c * P:(c + 1) * P, :], in_=fin[:]
                        )

    nc.compile()
    return nc


_NC_CACHE = {}


def _get_nc():
    if "nc" not in _NC_CACHE:
        _NC_CACHE["nc"] = build_nc()
    return _NC_CACHE["nc"]


def make_in_maps(inputs):
    """Shard the full inputs into 8 per-core input maps."""
    f = np.float32
    i = np.int32
    dsr_full = np.ascontiguousarray(
        np.asarray(inputs["dialog_sym_rep"], dtype=f).reshape(B, E)
    )
    kg = np.asarray(inputs["kg_adj"], dtype=f)
    rep = {
        "wg": np.ascontiguousarray(np.asarray(inputs["W_g"], dtype=f)),
        "wd1": np.ascontiguousarray(np.asarray(inputs["Wd1"], dtype=f)),
        "bd1": np.ascontiguousarray(np.asarray(inputs["bd1"], dtype=f)),
        "w1": np.ascontiguousarray(np.asarray(inputs["W1"], dtype=f)),
        "b1": np.ascontiguousarray(np.asarray(inputs["b1"], dtype=f)),
        "w2": np.ascontiguousarray(np.asarray(inputs["W2"], dtype=f)),
        "b2": np.ascontiguousarray(np.asarray(inputs["b2"], dtype=f)),
        "wm1": np.ascontiguousarray(np.asarray(inputs["Wm1"], dtype=f)),
        "bm1": np.ascontiguousarray(np.asarray(inputs["bm1"], dtype=f)),
        "wm2": np.ascontiguousarray(np.asarray(inputs["Wm2"], dtype=f)),
        "bm2": np.ascontiguousarray(np.asarray(inputs["bm2"], dtype=f)),
        "md": np.ascontiguousarray(np.asarray(inputs["m_d"], dtype=f)),
    }
    in_maps = []
    for c in range(N_CORES):
        bs = slice(c * B_LOC, (c + 1) * B_LOC)
        m = {
            "kga": np.ascontiguousarray(kg[c * E_LOC:(c + 1) * E_LOC, :]),
            "dsr": np.ascontiguousarray(dsr_full[bs]),
            "ds": np.ascontiguousarray(np.asarray(inputs["dialog_state"], dtype=i)[bs]),
            "symmask": np.ascontiguousarray(np.asarray(inputs["sym_mask"], dtype=i)[bs]),
            "symflag": np.ascontiguousarray(np.asarray(inputs["sym_flag"], dtype=i)[bs]),
            "sympmask": np.ascontiguousarray(
                np.asarray(inputs["symptoms_mask"], dtype=i)[bs]
            ),
            "dmask": np.ascontiguousarray(np.asarray(inputs["disease_mask"], dtype=i)[bs]),
        }
        m.update(rep)
        in_maps.append(m)
    return in_maps


def run(inputs, trace=False):
    nc = _get_nc()
    in_maps = make_in_maps(inputs)
    res = run_bass_kernel_spmd(nc, in_maps, list(range(N_CORES)), trace=trace)
    dd = np.concatenate([res.results[c]["out_dd"] for c in range(N_CORES)], axis=0)
    sf = np.concatenate([res.results[c]["out_sf"] for c in range(N_CORES)], axis=0)
    return (dd, sf), res


def kernel(**inputs):
    (dd, sf), _ = run(inputs)
    return dd, sf


def bench(inputs, reps=10):
    """Measure marginal per-exec wall time with a cached jitted runner and
    device-resident inputs (no NTFF profiling available under this axon env)."""
    import jax
    import numpy as _np
    from jax.sharding import Mesh, PartitionSpec
    from jax.experimental.shard_map import shard_map
    from concourse import bass2jax
    import concourse.mybir as _mybir
    import time as _time

    nc = _get_nc()
    in_maps = make_in_maps(inputs)
    bass2jax.install_neuronx_cc_hook()

    partition_name = nc.partition_id_tensor.name if nc.partition_id_tensor else None
    in_names, out_names, out_avals, zero_outs = [], [], [], []
    for alloc in nc.m.functions[0].allocations:
        if not isinstance(alloc, _mybir.MemoryLocationSet):
            continue
        name = alloc.memorylocations[0].name
        if alloc.kind == "ExternalInput":
            if name != partition_name:
                in_names.append(name)
        elif alloc.kind == "ExternalOutput":
            out_names.append(name)
            shape = tuple(alloc.tensor_shape)
            dtype = _mybir.dt.np(alloc.dtype)
            out_avals.append(jax.core.ShapedArray(shape, dtype))
            zero_outs.append(_np.zeros(shape, dtype))
    n_params = len(in_names)
    all_names = in_names + out_names

    def _body(*args):
        operands = list(args)
        if partition_name is not None:
            operands.append(bass2jax.partition_id_tensor())
        outs = bass2jax._bass_exec_p.bind(
            *operands,
            out_avals=tuple(out_avals),
            in_names=tuple(all_names + ([partition_name] if partition_name else [])),
            out_names=tuple(out_names),
            lowering_input_output_aliases=(),
            sim_require_finite=True,
            sim_require_nnan=True,
            nc=nc,
        )
        return tuple(outs)

    devices = jax.devices()[:N_CORES]
    mesh = Mesh(_np.asarray(devices), ("core",))
    n_out = len(out_names)
    sharded = jax.jit(
        shard_map(
            _body, mesh=mesh,
            in_specs=(PartitionSpec("core"),) * (n_params + n_out),
            out_specs=(PartitionSpec("core"),) * n_out,
            check_rep=False,
        ),
        keep_unused=True,
    )
    concat_in = [
        _np.concatenate([in_maps[c][nm] for c in range(N_CORES)], axis=0)
        for nm in in_names
    ]
    concat_zero = [
        _np.zeros((N_CORES * z.shape[0], *z.shape[1:]), z.dtype) for z in zero_outs
    ]
    args = [jax.device_put(a) for a in concat_in + concat_zero]
    # warmup (includes compile)
    out = sharded(*args)
    jax.block_until_ready(out)
    times = []
    for _ in range(reps):
        t0 = _time.perf_counter()
        out = sharded(*args)
        jax.block_until_ready(out)
        times.append(_time.perf_counter() - t0)
    return times, out, out_names
